# revision 10
# baseline (speedup 1.0000x reference)
"""CrossAttention kernel for 8 TRN2 NeuronCores (Bass/Tile).

Reference computation (per batch b):
    q = x @ Wq ; k = ctx @ Wk ; v = ctx @ Wv        (heads H=8, dh=64)
    attn = softmax(q k^T / sqrt(dh)) ; o = attn @ v
    out = o @ Wo + bo

Sharding (8 cores): core c -> (batch b = c//2, head-group hg = c%2).
Each core handles 4 heads of one batch over the full sequence; the two
head-group partial outputs per batch are summed on the host (Wo is
sliced by rows, so partials add exactly).

Layout strategy: everything on-chip is kept "feature-major" (transposed)
so no on-chip transpose is ever needed:
  - host passes xq=[128,4,512]x4 quarters, same for context (fp16,
    pre-swizzled on host so every DMA descriptor is a long contiguous run)
  - QT = Wq^T x^T, KT = Wk^T c^T  (d on partitions, seq on free)
  - V  = c @ Wv natural            (seq on partitions, d on free)
  - S^T = K_h Q_h^T                (keys m on partitions, queries n free)
  - E = exp(S^T/8)  (ScalarE, read PSUM directly; logits are tiny so no
    max-subtraction is needed -- softmax is shift-invariant)
  - O' = [V_h | ones]-stationary AV matmul: rows 0..63 give O'^T, rows
    64..127 give the softmax denominators pre-broadcast for normalize
  - O^T normalized with 1/sums, concatenated; Y^T = Wo_s^T O^T

Scheduling: the inner attention loop is software-pipelined so the PE
always has the NEXT S matmul queued ahead of the current AV matmul
(which waits on ACT's exp). All projections are prefetched 1+ m-tiles
ahead. PSUM->SBUF copies + normalize run on DVE; exp on ACT; matmuls
on PE; Vp-ones memset on DVE at t=0. Engines overlap.
"""

import os

import numpy as np

import concourse.bass as bass
import concourse.mybir as mybir
import concourse.tile as tile
from concourse import bacc
from concourse.bass_utils import run_bass_kernel_spmd

F16 = mybir.dt.float16
F32 = mybir.dt.float32

D = 512          # model dim
N = 2048         # query seq len
M = 2048         # key seq len
HPC = 4          # heads per core
DH = 64          # head dim
DS = HPC * DH    # per-core inner dim = 256
SCALE = 1.0 / 8.0  # 1/sqrt(64)
P = 128
KT_D = D // P    # 4 k-tiles over model dim
NS = 512         # query sub-chunk / seq quarter

_NP16 = np.float16


def _build_nc():
    nc = bacc.Bacc(None, target_bir_lowering=False)

    xq = [nc.declare_dram_parameter(f"xq{i}", [P, KT_D, NS], F16,
                                    isOutput=False) for i in range(4)]
    cq = [nc.declare_dram_parameter(f"cq{i}", [P, KT_D, NS], F16,
                                    isOutput=False) for i in range(4)]
    wq = nc.declare_dram_parameter("wq", [P, KT_D, DS], F16, isOutput=False)
    wk = nc.declare_dram_parameter("wk", [P, KT_D, DS], F16, isOutput=False)
    wv = nc.declare_dram_parameter("wv", [P, KT_D, DS], F16, isOutput=False)
    wo = nc.declare_dram_parameter("wo", [P, DS // P, D], F16, isOutput=False)
    yT = nc.declare_dram_parameter("yT", [N // NS, P, D // P, NS], F16,
                                   isOutput=True)

    with tile.TileContext(nc) as tc:
        _emit(tc, xq, cq, wq, wk, wv, wo, yT)
    nc.finalize()
    return nc


def _emit(tc, xq, cq, wq, wk, wv, wo, yT):
    nc = tc.nc
    MT = M // P          # 16 m-tiles over keys
    NCH = 2 * NS         # psum tile width for a head-pair
    EXP = mybir.ActivationFunctionType.Exp

    from contextlib import ExitStack

    with ExitStack() as ctx:
        const = ctx.enter_context(tc.tile_pool(name="const", bufs=1))
        work = ctx.enter_context(tc.tile_pool(name="work", bufs=4))
        rcp = ctx.enter_context(tc.tile_pool(name="rcp", bufs=2))
        yout = ctx.enter_context(tc.tile_pool(name="yout", bufs=2))
        ps_s = ctx.enter_context(tc.tile_pool(name="ps_s", bufs=2, space="PSUM"))
        ps_o = ctx.enter_context(tc.tile_pool(name="ps_o", bufs=2, space="PSUM"))
        ps_v = ctx.enter_context(tc.tile_pool(name="ps_v", bufs=1, space="PSUM"))
        ps_p = ctx.enter_context(tc.tile_pool(name="ps_p", bufs=1, space="PSUM"))

        # ---- resident SBUF tensors ----
        xT_q = [const.tile([P, KT_D, NS], F16, name=f"xT{i}") for i in range(4)]
        cT_q = [const.tile([P, KT_D, NS], F16, name=f"cT{i}") for i in range(4)]
        wq_sb = const.tile([P, KT_D, DS], F16)
        wk_sb = const.tile([P, KT_D, DS], F16)
        wv_sb = const.tile([P, KT_D, DS], F16)
        wo_sb = const.tile([P, DS // P, D], F16)
        QT_sb = const.tile([P, DS // P, N], F16)
        KT_sb = const.tile([P, DS // P, M], F16)
        # per (m-tile, head): 128 stationary columns = [V_h (64) | ones (64)]
        # so one matmul yields O'^T rows 0..63 AND the softmax sums
        # replicated on rows 64..127 (pre-broadcast for the normalize).
        Vp_sb = const.tile([P, MT, HPC, P], F16)
        Ocat = const.tile([P, DS // P, N], F16)

        # ---- input DMAs, ramp-ordered ----
        nc.vector.memset(Vp_sb[:, :, :, DH:P], 1.0)
        nc.sync.dma_start(wq_sb[:], wq[:])
        nc.sync.dma_start(xT_q[0][:], xq[0][:])
        nc.gpsimd.dma_start(wk_sb[:], wk[:])
        nc.gpsimd.dma_start(cT_q[0][:], cq[0][:])
        nc.gpsimd.dma_start(wv_sb[:], wv[:])
        nc.sync.dma_start(xT_q[1][:], xq[1][:])
        nc.gpsimd.dma_start(cT_q[1][:], cq[1][:])
        nc.sync.dma_start(xT_q[2][:], xq[2][:])
        nc.gpsimd.dma_start(cT_q[2][:], cq[2][:])
        nc.sync.dma_start(xT_q[3][:], xq[3][:])
        nc.gpsimd.dma_start(cT_q[3][:], cq[3][:])
        nc.sync.dma_start(wo_sb[:], wo[:])

        # ---- projections, prefetched ahead of use ----
        proj_done = set()

        def emit_q(pr, ch):
            if ("q", pr, ch) in proj_done:
                return
            proj_done.add(("q", pr, ch))
            ps = ps_p.tile([P, NS], F32, tag="psp", name=f"q{pr}{ch}")
            for kt in range(KT_D):
                nc.tensor.matmul(
                    ps[:, :NS],
                    lhsT=wq_sb[:, kt, pr * P:(pr + 1) * P],
                    rhs=xT_q[ch][:, kt, :],
                    start=(kt == 0),
                    stop=(kt == KT_D - 1),
                )
            nc.vector.tensor_copy(
                QT_sb[:, pr, ch * NS:(ch + 1) * NS], ps[:, :NS]
            )

        def emit_k(pr, dt):
            if ("k", pr, dt) in proj_done:
                return
            proj_done.add(("k", pr, dt))
            ps = ps_p.tile([P, NS], F32, tag="psp", name=f"k{pr}{dt}")
            for kt in range(KT_D):
                nc.tensor.matmul(
                    ps[:, :NS],
                    lhsT=wk_sb[:, kt, pr * P:(pr + 1) * P],
                    rhs=cT_q[dt][:, kt, :],
                    start=(kt == 0),
                    stop=(kt == KT_D - 1),
                )
            nc.vector.tensor_copy(
                KT_sb[:, pr, dt * NS:(dt + 1) * NS], ps[:, :NS]
            )

        def emit_v(mt):
            if ("v", mt) in proj_done:
                return
            proj_done.add(("v", mt))
            ps = ps_v.tile([P, NS], F32, tag="pv", name=f"v{mt}")
            for kt in range(KT_D):
                nc.tensor.matmul(
                    ps[:, :DS],
                    lhsT=cT_q[mt // 4][:, kt, (mt % 4) * P:(mt % 4 + 1) * P],
                    rhs=wv_sb[:, kt, :],
                    start=(kt == 0),
                    stop=(kt == KT_D - 1),
                )
            nc.vector.tensor_copy(
                Vp_sb[:, mt, :, 0:DH],
                ps[:, 0:DS].rearrange("p (h d) -> p h d", h=HPC),
            )

        def emit_y(nch):
            yt = yout.tile([P, D // P, NS], F16, tag="y")
            for dt4 in range(D // P):
                ps = ps_p.tile([P, NS], F32, tag="psp", name=f"y{nch}{dt4}")
                for kt in range(DS // P):
                    nc.tensor.matmul(
                        ps[:, :NS],
                        lhsT=wo_sb[:, kt, dt4 * P:(dt4 + 1) * P],
                        rhs=Ocat[:, kt, nch * NS:(nch + 1) * NS],
                        start=(kt == 0),
                        stop=(kt == DS // P - 1),
                    )
                nc.vector.tensor_copy(yt[:, dt4, :], ps[:, :NS])
            nc.sync.dma_start(yT[nch], yt[:])

        # ---- software-pipelined attention ----
        blocks = [(nch, pr) for nch in range(N // NS) for pr in range(HPC // 2)]
        NB = len(blocks)

        st_tiles = {}

        def emit_S(bi, mt):
            nch, pr = blocks[bi]
            n0 = nch * NS
            h0, h1 = 2 * pr, 2 * pr + 1
            emit_q(pr, nch)
            emit_k(pr, mt // 4)
            st = ps_s.tile([P, NCH], F32, tag="ps", name=f"s{bi}_{mt}")
            for i, h in enumerate((h0, h1)):
                dp = (h % 2) * DH
                nc.tensor.matmul(
                    st[:, i * NS:(i + 1) * NS],
                    lhsT=KT_sb[dp:dp + DH, pr, mt * P:(mt + 1) * P],
                    rhs=QT_sb[dp:dp + DH, pr, n0:n0 + NS],
                    start=True,
                    stop=True,
                )
            st_tiles[(bi, mt)] = st

        # prologue: first block's first S, plus the first two V tiles
        emit_S(0, 0)
        emit_v(0)
        emit_v(1)

        for bi, (nch, pr) in enumerate(blocks):
            h0, h1 = 2 * pr, 2 * pr + 1
            po = [
                ps_o.tile([P, NS], F32, tag="po", name=f"po{bi}_{i}")
                for i in range(2)
            ]
            for mt in range(MT):
                # exp of the current S tile: most m-tiles on ACT (table
                # exp); every third on DVE via the fp16 Schraudolph trick
                # (exp(x) ~= bitcast_fp16(round(1024*log2(e)*x + b)); the
                # piecewise-linear 2^frac error is ~1.5% rms on those
                # tiles, which the softmax normalization tolerates).
                st = st_tiles.pop((bi, mt))
                e = work.tile([P, NCH], F16, tag="e")
                if mt % 3 == 2:
                    nc.vector.tensor_scalar(
                        e.bitcast(mybir.dt.int16)[:], st[:],
                        1024.0 * 1.4426950408889634 * SCALE, 15315.5,
                        mybir.AluOpType.mult, mybir.AluOpType.add,
                    )
                else:
                    nc.scalar.activation(e[:], st[:], EXP, scale=SCALE)

                # PE: prefetch projections + next S BEFORE the AV that
                # waits on ACT, so the PE never idles on the exp.
                if bi == 0:
                    if mt + 2 < MT:
                        emit_v(mt + 2)
                    if mt in (1, 3, 5, 7):        # K for head-pair 1
                        emit_k(1, (mt - 1) // 2)
                    elif mt == 9:
                        emit_q(1, 0)
                    elif mt == 11:
                        emit_q(0, 1)
                elif mt == 2 and bi + 2 < NB:     # Q for the block after next
                    emit_q(blocks[bi + 2][1], blocks[bi + 2][0])
                if mt + 1 < MT:
                    emit_S(bi, mt + 1)
                elif bi + 1 < NB:
                    emit_S(bi + 1, 0)

                # PE: accumulate O' and softmax sums for both heads
                for i, h in enumerate((h0, h1)):
                    nc.tensor.matmul(
                        po[i][:],
                        lhsT=Vp_sb[:, mt, h, :],
                        rhs=e[:, i * NS:(i + 1) * NS],
                        start=(mt == 0),
                        stop=(mt == MT - 1),
                    )

            # DVE: normalize O^T with 1/sums (sums sit pre-broadcast on
            # rows 64..127 of po).
            n0 = nch * NS
            for i, h in enumerate((h0, h1)):
                dp = (h % 2) * DH
                sc = rcp.tile([DH, NS], F32, tag="sc")
                nc.vector.tensor_copy(sc[:], po[i][DH:P, :])
                rc = rcp.tile([DH, NS], F32, tag="rc")
                nc.vector.reciprocal_approx_fast(rc[:], sc[:])
                nc.vector.tensor_tensor(
                    Ocat[dp:dp + DH, pr, n0:n0 + NS],
                    po[i][0:DH, :],
                    rc[:],
                    mybir.AluOpType.mult,
                )

            # output projection for this n-chunk once both head-pairs done
            if pr == 1:
                emit_y(nch)


def _install_ntff_hook():
    """Best-effort NTFF profiling under axon: provide the antenv.axon_hooks
    shim the boot code looks for, and avoid the artifact upload."""
    try:
        import sys
        import types

        import concourse.bass_utils as bu

        bu.upload_artifacts = lambda d: d  # no S3 in this sandbox
        try:
            from antenv.axon_hooks import get_axon_ntff_profile_hook  # noqa: F401
            return  # already present
        except ImportError:
            pass
        import antenv
        from trn_agent_boot.trn_boot import _ntff_profile_via_ctypes

        mod = types.ModuleType("antenv.axon_hooks")
        _state = {"hook": _ntff_profile_via_ctypes("/opt/axon/libaxon_pjrt.so")}
        mod.set_axon_ntff_profile_hook = lambda h: _state.__setitem__("hook", h)
        mod.get_axon_ntff_profile_hook = lambda: _state["hook"]
        sys.modules["antenv.axon_hooks"] = mod
        antenv.axon_hooks = mod
    except Exception as e:  # pragma: no cover
        print(f"ntff hook install failed ({e}); running without trace")


def _swizzle_dn(a):
    """[D-like, n] -> [128, D/128, n] contiguous (partition-major)."""
    d, n = a.shape
    return np.ascontiguousarray(
        a.reshape(d // P, P, n).transpose(1, 0, 2)).astype(_NP16)


def kernel(x, context, Wq, Wk, Wv, Wo, bo):
    x = np.asarray(x, dtype=np.float32)
    context = np.asarray(context, dtype=np.float32)
    Wq = np.asarray(Wq, dtype=np.float32)
    Wk = np.asarray(Wk, dtype=np.float32)
    Wv = np.asarray(Wv, dtype=np.float32)
    Wo = np.asarray(Wo, dtype=np.float32)
    bo = np.asarray(bo, dtype=np.float32)
    B = x.shape[0]

    in_maps = []
    for c in range(8):
        b, hg = c // 2, c % 2
        sl = slice(hg * DS, (hg + 1) * DS)
        xT = _swizzle_dn(x[b].T)        # [128, 4, 2048]
        cT = _swizzle_dn(context[b].T)
        m = {
            "wq": _swizzle_dn(Wq[:, sl]),
            "wk": _swizzle_dn(Wk[:, sl]),
            "wv": _swizzle_dn(Wv[:, sl]),
            "wo": _swizzle_dn(Wo[sl, :]),
        }
        for i in range(4):
            m[f"xq{i}"] = np.ascontiguousarray(xT[:, :, i * NS:(i + 1) * NS])
            m[f"cq{i}"] = np.ascontiguousarray(cT[:, :, i * NS:(i + 1) * NS])
        in_maps.append(m)

    nc = _build_nc()
    trace = bool(int(os.environ.get("BASS_KERNEL_TRACE", "0")))
    if trace:
        _install_ntff_hook()
    res = run_bass_kernel_spmd(nc, in_maps, list(range(8)), trace=trace)
    if trace and res.exec_time_ns is not None:
        print(f"HW exec time: {res.exec_time_ns} ns")

    out = np.empty((B, N, D), dtype=np.float32)
    for b in range(B):
        # yT: [nch, p, dt, ns] -> y[d, n] with d = dt*128+p, n = nch*512+ns
        yt = (res.results[2 * b]["yT"].astype(np.float32)
              + res.results[2 * b + 1]["yT"].astype(np.float32))
        y = yt.transpose(2, 1, 0, 3).reshape(D, N)
        out[b] = y.T + bo[None, :]
    return out
